# revision 9
# baseline (speedup 1.0000x reference)
"""Trainium2 Bass kernel for ConvertDubaiMasks.

Maps each RGB pixel of a (3, 4096, 4096) f32 image (integer values 0..255)
to a palette class id (uint8). A pixel matches palette color i iff all three
channels match; matched pixels output class_ids[i], unmatched output 0.

Implementation: pack key = R*65536 + G*256 + B (exact in f32, < 2^24),
compare against the packed palette colors with nonzero class ids
(zero-class colors contribute nothing), and sum class_id * (key == k_i).

Sharding: H split across 8 NeuronCores (512 rows each), SPMD, no
communication.
"""

import numpy as np

H = 4096
W = 4096
N_CORES = 8
H_PER = H // N_CORES  # 512
P = 128  # SBUF partitions

# Set by test harness to capture a profile; results stashed in LAST_RESULTS.
PROFILE = False
LAST_RESULTS = None

# Use the fused custom-DVE ops (2 DVE passes for the 5-way palette match)
# instead of 5 tensor_scalar + 4 tensor_tensor. Falls back automatically if
# the palette shape doesn't fit the fused formulation.
USE_CUSTOM = True

_CUSTOM_OPS = None
_CUSTOM_TRIED = False


def _register_custom_ops():
    """Register two fused DVE ops for the palette match:

    DUBAI_M123_ANT:  out = 1*eq(x,C0) + 2*eq(x,C1) + 3*eq(x,C2)
    DUBAI_M45A_ANT:  out = Src1 + 4*eq(x,C0) + 5*eq(x,C1)

    Multipliers are encoded structurally (shared-subexpression add chains)
    since Spec bodies cannot contain literals. Returns (opA, opB) or None.
    """
    global _CUSTOM_OPS, _CUSTOM_TRIED
    if _CUSTOM_TRIED:
        return _CUSTOM_OPS
    _CUSTOM_TRIED = True
    try:
        import concourse.dve_ops as dve_ops
        from concourse.dve_ops import DveOp
        from concourse.dve_spec import (
            C0,
            C1,
            C2,
            Spec,
            Src0,
            Src1,
            _has_src1,
            eq,
            lower,
        )
        from concourse.dve_uop import DveOpSpec

        e1, e2, e3 = eq(Src0, C0), eq(Src0, C1), eq(Src0, C2)
        t = e2 + e3
        body_a = ((e1 + t) + t) + e3  # e1 + 2*e2 + 3*e3

        def ref_a(in0, in1, s0, s1, imm2):
            return (
                (in0 == s0).astype(np.float32)
                + 2.0 * (in0 == s1).astype(np.float32)
                + 3.0 * (in0 == imm2).astype(np.float32)
            ).astype(np.float32)

        f4, f5 = eq(Src0, C0), eq(Src0, C1)
        u = f4 + f5
        body_b = ((((Src1 + u) + u) + u) + u) + f5  # Src1 + 4*e4 + 5*e5

        def ref_b(in0, in1, s0, s1, imm2):
            return (
                in1
                + 4.0 * (in0 == s0).astype(np.float32)
                + 5.0 * (in0 == s1).astype(np.float32)
            ).astype(np.float32)

        ops = []
        for nm, body, ref in (
            ("DUBAI_M123_ANT", body_a, ref_a),
            ("DUBAI_M45A_ANT", body_b, ref_b),
        ):
            if nm in dve_ops._SUB_OPCODE_FOR_NAME:
                ops.append(next(o for o in dve_ops.OPS if o.name == nm))
                continue
            row = max(dve_ops._SUB_OPCODE_FOR_NAME.values()) + 1
            assert row < 0x20, "custom-DVE opcode rows exhausted"
            spec = Spec(body=body, reference=ref)
            shas = {}
            for ver in ("v3", "v4"):
                try:
                    s = DveOpSpec(
                        name=nm, opcode=row, uops=lower(spec, ver=ver),
                        rd1_en=_has_src1(spec),
                    )
                    shas[ver] = s.sha(ver)
                except Exception:
                    pass
            if "v3" not in shas:
                raise RuntimeError(f"{nm}: v3 lowering failed")
            op = DveOp(nm, spec, subdim=False, uops_sha=shas)
            dve_ops._SUB_OPCODE_FOR_NAME[nm] = row
            dve_ops.OPS.append(op)
            dve_ops.CUSTOM_DVE_SPECS[nm] = spec
            ops.append(op)
        _CUSTOM_OPS = tuple(ops)
    except Exception:
        _CUSTOM_OPS = None
    return _CUSTOM_OPS


def _emit(tc, x_ap, y_ap, terms, h_per, w, wchunk, key_engine1="gpsimd"):
    """Emit the per-core tile program.

    x_ap: (3, h_per, w) f32 DRAM input; y_ap: (h_per, w) u8 DRAM output.
    terms: list of (packed_key_float, class_id_float), class_id != 0.
    """
    import concourse.mybir as mybir

    nc = tc.nc
    f32 = mybir.dt.float32
    bf16 = mybir.dt.bfloat16
    u8 = mybir.dt.uint8
    Alu = mybir.AluOpType

    n_row = h_per // P
    n_col = w // wchunk

    with (
        tc.tile_pool(name="io", bufs=3) as io_pool,
        tc.tile_pool(name="tmp", bufs=2) as tmp_pool,
    ):
        for r in range(n_row):
            for c in range(n_col):
                rows = slice(r * P, (r + 1) * P)
                cols = slice(c * wchunk, (c + 1) * wchunk)

                # One DMA for all 3 planes: (3, 128, wchunk) -> [128, 3, wchunk]
                xt = io_pool.tile([P, 3, wchunk], f32, tag="x")
                nc.sync.dma_start(
                    xt[:], x_ap[:, rows, cols].rearrange("c p w -> p c w")
                )
                Rt, Gt, Bt = xt[:, 0, :], xt[:, 1, :], xt[:, 2, :]

                out_t = io_pool.tile([P, wchunk], u8, tag="out")

                # key = R*65536 + (G*256 + B), built in place over G then R.
                # First pack on GPSIMD (otherwise idle) to unload the DVE;
                # Pool has no scalar_tensor_tensor, so mul then add.
                if key_engine1 == "gpsimd":
                    nc.gpsimd.tensor_scalar_mul(Gt, Gt, 256.0)
                    nc.gpsimd.tensor_tensor(Gt, Gt, Bt, Alu.add)
                else:
                    nc.vector.scalar_tensor_tensor(
                        Gt, Gt, 256.0, Bt, Alu.mult, Alu.add
                    )
                nc.vector.scalar_tensor_tensor(
                    Rt, Rt, 65536.0, Gt, Alu.mult, Alu.add
                )
                key = Rt

                custom = None
                if USE_CUSTOM and [int(c) for _, c in sorted(terms, key=lambda t: t[1])] == [1, 2, 3, 4, 5]:
                    custom = _register_custom_ops()

                if custom is not None:
                    st = sorted(terms, key=lambda t: t[1])
                    op_a, op_b = custom
                    acc = tmp_pool.tile([P, wchunk], bf16, tag="acc")
                    nc.vector._custom_dve(
                        op_a, out=acc[:], in0=key,
                        s0=st[0][0], s1=st[1][0], imm2=st[2][0],
                    )
                    nc.vector._custom_dve(
                        op_b, out=out_t[:], in0=key, in1=acc[:],
                        s0=st[3][0], s1=st[4][0],
                    )
                elif not terms:
                    nc.vector.memset(out_t[:], 0)
                elif len(terms) == 1:
                    k0, cid0 = terms[0]
                    nc.vector.tensor_scalar(
                        out_t[:], key, k0, cid0, Alu.is_equal, Alu.mult
                    )
                else:
                    acc = tmp_pool.tile([P, wchunk], bf16, tag="acc")
                    k0, cid0 = terms[0]
                    nc.vector.tensor_scalar(
                        acc[:], key, k0, cid0, Alu.is_equal, Alu.mult
                    )
                    for j, (kj, cidj) in enumerate(terms[1:]):
                        last = j == len(terms) - 2
                        e = tmp_pool.tile([P, wchunk], bf16, tag="e")
                        nc.vector.tensor_scalar(
                            e[:], key, kj, cidj, Alu.is_equal, Alu.mult
                        )
                        if last:
                            nc.vector.tensor_tensor(out_t[:], acc[:], e[:], Alu.add)
                        else:
                            nc.vector.tensor_tensor(acc[:], acc[:], e[:], Alu.add)

                nc.sync.dma_start(y_ap[rows, cols], out_t[:])


def _terms_from_palette(colors, class_ids):
    colors = np.asarray(colors).astype(np.int64)
    class_ids = np.asarray(class_ids).astype(np.int64)
    terms = []
    for i in range(colors.shape[0]):
        cid = int(class_ids[i])
        if cid == 0:
            continue
        k = int(colors[i, 0]) * 65536 + int(colors[i, 1]) * 256 + int(colors[i, 2])
        terms.append((float(k), float(cid)))
    return terms


_NC_CACHE = {}


def _build_nc(terms, h_per=H_PER, w=W, wchunk=2048, key_engine1="gpsimd"):
    key = (tuple(terms), h_per, w, wchunk, key_engine1)
    if key in _NC_CACHE:
        return _NC_CACHE[key]

    import concourse.bacc as bacc
    import concourse.mybir as mybir
    from concourse.tile import TileContext

    nc = bacc.Bacc()
    x = nc.dram_tensor("x", (3, h_per, w), mybir.dt.float32, kind="ExternalInput")
    y = nc.dram_tensor("y", (h_per, w), mybir.dt.uint8, kind="ExternalOutput")

    with TileContext(nc) as tc:
        _emit(tc, x.ap(), y.ap(), terms, h_per, w, wchunk, key_engine1)

    nc.compile()
    _NC_CACHE[key] = nc
    return nc


def kernel(tensor, colors, class_ids):
    global LAST_RESULTS
    from concourse.bass_utils import run_bass_kernel_spmd

    tensor = np.asarray(tensor, dtype=np.float32)
    terms = _terms_from_palette(colors, class_ids)
    nc = _build_nc(terms)

    in_maps = [
        {"x": np.ascontiguousarray(tensor[:, k * H_PER : (k + 1) * H_PER, :])}
        for k in range(N_CORES)
    ]
    results = run_bass_kernel_spmd(
        nc, in_maps, core_ids=list(range(N_CORES)), trace=PROFILE
    )
    LAST_RESULTS = results
    return np.concatenate([results.results[k]["y"] for k in range(N_CORES)], axis=0)


# revision 10
# speedup vs baseline: 3.1898x; 3.1898x over previous
"""Trainium2 Bass kernel for ConvertDubaiMasks.

Maps each RGB pixel of a (3, 4096, 4096) f32 image (integer values 0..255)
to a palette class id (uint8). A pixel matches palette color i iff all three
channels match; matched pixels output class_ids[i], unmatched output 0.

Implementation: pack key = R*65536 + G*256 + B (exact in f32, < 2^24),
compare against the packed palette colors with nonzero class ids
(zero-class colors contribute nothing), and sum class_id * (key == k_i).

Sharding: H split across 8 NeuronCores (512 rows each), SPMD, no
communication.
"""

import numpy as np

H = 4096
W = 4096
N_CORES = 8
H_PER = H // N_CORES  # 512
P = 128  # SBUF partitions

# Set by test harness to capture a profile; results stashed in LAST_RESULTS.
PROFILE = False
LAST_RESULTS = None

# Use the fused custom-DVE ops (2 DVE passes for the 5-way palette match)
# instead of 5 tensor_scalar + 4 tensor_tensor. Falls back automatically if
# the palette shape doesn't fit the fused formulation.
USE_CUSTOM = True

_CUSTOM_OPS = None
_CUSTOM_TRIED = False


def _register_custom_ops():
    """Register two fused DVE ops for the palette match:

    DUBAI_M123_ANT:  out = 1*eq(x,C0) + 2*eq(x,C1) + 3*eq(x,C2)
    DUBAI_M45A_ANT:  out = Src1 + 4*eq(x,C0) + 5*eq(x,C1)

    Multipliers are encoded structurally (shared-subexpression add chains)
    since Spec bodies cannot contain literals. Returns (opA, opB) or None.
    """
    global _CUSTOM_OPS, _CUSTOM_TRIED
    if _CUSTOM_TRIED:
        return _CUSTOM_OPS
    _CUSTOM_TRIED = True
    try:
        import concourse.dve_ops as dve_ops
        from concourse.dve_ops import DveOp
        from concourse.dve_spec import (
            C0,
            C1,
            C2,
            Spec,
            Src0,
            Src1,
            _has_src1,
            eq,
            lower,
        )
        from concourse.dve_uop import DveOpSpec

        e1, e2, e3 = eq(Src0, C0), eq(Src0, C1), eq(Src0, C2)
        t = e2 + e3
        body_a = ((e1 + t) + t) + e3  # e1 + 2*e2 + 3*e3

        def ref_a(in0, in1, s0, s1, imm2):
            return (
                (in0 == s0).astype(np.float32)
                + 2.0 * (in0 == s1).astype(np.float32)
                + 3.0 * (in0 == imm2).astype(np.float32)
            ).astype(np.float32)

        f4, f5 = eq(Src0, C0), eq(Src0, C1)
        u = f4 + f5
        body_b = ((((Src1 + u) + u) + u) + u) + f5  # Src1 + 4*e4 + 5*e5

        def ref_b(in0, in1, s0, s1, imm2):
            return (
                in1
                + 4.0 * (in0 == s0).astype(np.float32)
                + 5.0 * (in0 == s1).astype(np.float32)
            ).astype(np.float32)

        ops = []
        for nm, body, ref in (
            ("DUBAI_M123_ANT", body_a, ref_a),
            ("DUBAI_M45A_ANT", body_b, ref_b),
        ):
            if nm in dve_ops._SUB_OPCODE_FOR_NAME:
                ops.append(next(o for o in dve_ops.OPS if o.name == nm))
                continue
            row = max(dve_ops._SUB_OPCODE_FOR_NAME.values()) + 1
            assert row < 0x20, "custom-DVE opcode rows exhausted"
            spec = Spec(body=body, reference=ref)
            shas = {}
            for ver in ("v3", "v4"):
                try:
                    s = DveOpSpec(
                        name=nm, opcode=row, uops=lower(spec, ver=ver),
                        rd1_en=_has_src1(spec),
                    )
                    shas[ver] = s.sha(ver)
                except Exception:
                    pass
            if "v3" not in shas:
                raise RuntimeError(f"{nm}: v3 lowering failed")
            op = DveOp(nm, spec, subdim=False, uops_sha=shas)
            dve_ops._SUB_OPCODE_FOR_NAME[nm] = row
            dve_ops.OPS.append(op)
            dve_ops.CUSTOM_DVE_SPECS[nm] = spec
            ops.append(op)
        _CUSTOM_OPS = tuple(ops)
    except Exception:
        _CUSTOM_OPS = None
    return _CUSTOM_OPS


def _emit(tc, x_ap, y_ap, terms, h_per, w, wchunk, key_engine1="vector"):
    """Emit the per-core tile program.

    x_ap: (3, h_per, w) f32 DRAM input; y_ap: (h_per, w) u8 DRAM output.
    terms: list of (packed_key_float, class_id_float), class_id != 0.
    """
    import concourse.mybir as mybir

    nc = tc.nc
    f32 = mybir.dt.float32
    bf16 = mybir.dt.bfloat16
    u8 = mybir.dt.uint8
    Alu = mybir.AluOpType

    n_row = h_per // P
    n_col = w // wchunk

    with (
        tc.tile_pool(name="io", bufs=3) as io_pool,
        tc.tile_pool(name="tmp", bufs=2) as tmp_pool,
    ):
        for r in range(n_row):
            for c in range(n_col):
                rows = slice(r * P, (r + 1) * P)
                cols = slice(c * wchunk, (c + 1) * wchunk)

                # One DMA for all 3 planes: (3, 128, wchunk) -> [128, 3, wchunk]
                xt = io_pool.tile([P, 3, wchunk], f32, tag="x")
                nc.sync.dma_start(
                    xt[:], x_ap[:, rows, cols].rearrange("c p w -> p c w")
                )
                Rt, Gt, Bt = xt[:, 0, :], xt[:, 1, :], xt[:, 2, :]

                out_t = io_pool.tile([P, wchunk], u8, tag="out")

                # key = R*65536 + (G*256 + B), built in place over G then R.
                # First pack on GPSIMD (otherwise idle) to unload the DVE;
                # Pool has no scalar_tensor_tensor, so mul then add.
                if key_engine1 == "gpsimd":
                    nc.gpsimd.tensor_scalar_mul(Gt, Gt, 256.0)
                    nc.gpsimd.tensor_tensor(Gt, Gt, Bt, Alu.add)
                else:
                    nc.vector.scalar_tensor_tensor(
                        Gt, Gt, 256.0, Bt, Alu.mult, Alu.add
                    )
                nc.vector.scalar_tensor_tensor(
                    Rt, Rt, 65536.0, Gt, Alu.mult, Alu.add
                )
                key = Rt

                custom = None
                if USE_CUSTOM and [int(c) for _, c in sorted(terms, key=lambda t: t[1])] == [1, 2, 3, 4, 5]:
                    custom = _register_custom_ops()

                if custom is not None:
                    st = sorted(terms, key=lambda t: t[1])
                    op_a, op_b = custom
                    acc = tmp_pool.tile([P, wchunk], bf16, tag="acc")
                    nc.vector._custom_dve(
                        op_a, out=acc[:], in0=key,
                        s0=st[0][0], s1=st[1][0], imm2=st[2][0],
                    )
                    nc.vector._custom_dve(
                        op_b, out=out_t[:], in0=key, in1=acc[:],
                        s0=st[3][0], s1=st[4][0],
                    )
                elif not terms:
                    nc.vector.memset(out_t[:], 0)
                elif len(terms) == 1:
                    k0, cid0 = terms[0]
                    nc.vector.tensor_scalar(
                        out_t[:], key, k0, cid0, Alu.is_equal, Alu.mult
                    )
                else:
                    acc = tmp_pool.tile([P, wchunk], bf16, tag="acc")
                    k0, cid0 = terms[0]
                    nc.vector.tensor_scalar(
                        acc[:], key, k0, cid0, Alu.is_equal, Alu.mult
                    )
                    for j, (kj, cidj) in enumerate(terms[1:]):
                        last = j == len(terms) - 2
                        e = tmp_pool.tile([P, wchunk], bf16, tag="e")
                        nc.vector.tensor_scalar(
                            e[:], key, kj, cidj, Alu.is_equal, Alu.mult
                        )
                        if last:
                            nc.vector.tensor_tensor(out_t[:], acc[:], e[:], Alu.add)
                        else:
                            nc.vector.tensor_tensor(acc[:], acc[:], e[:], Alu.add)

                nc.sync.dma_start(y_ap[rows, cols], out_t[:])


def _terms_from_palette(colors, class_ids):
    colors = np.asarray(colors).astype(np.int64)
    class_ids = np.asarray(class_ids).astype(np.int64)
    terms = []
    for i in range(colors.shape[0]):
        cid = int(class_ids[i])
        if cid == 0:
            continue
        k = int(colors[i, 0]) * 65536 + int(colors[i, 1]) * 256 + int(colors[i, 2])
        terms.append((float(k), float(cid)))
    return terms


_NC_CACHE = {}


def _build_nc(terms, h_per=H_PER, w=W, wchunk=2048, key_engine1="vector"):
    key = (tuple(terms), h_per, w, wchunk, key_engine1)
    if key in _NC_CACHE:
        return _NC_CACHE[key]

    import concourse.bacc as bacc
    import concourse.mybir as mybir
    from concourse.tile import TileContext

    nc = bacc.Bacc()
    x = nc.dram_tensor("x", (3, h_per, w), mybir.dt.float32, kind="ExternalInput")
    y = nc.dram_tensor("y", (h_per, w), mybir.dt.uint8, kind="ExternalOutput")

    with TileContext(nc) as tc:
        _emit(tc, x.ap(), y.ap(), terms, h_per, w, wchunk, key_engine1)

    nc.compile()
    _NC_CACHE[key] = nc
    return nc


def kernel(tensor, colors, class_ids):
    global LAST_RESULTS
    from concourse.bass_utils import run_bass_kernel_spmd

    tensor = np.asarray(tensor, dtype=np.float32)
    terms = _terms_from_palette(colors, class_ids)
    nc = _build_nc(terms)

    in_maps = [
        {"x": np.ascontiguousarray(tensor[:, k * H_PER : (k + 1) * H_PER, :])}
        for k in range(N_CORES)
    ]
    results = run_bass_kernel_spmd(
        nc, in_maps, core_ids=list(range(N_CORES)), trace=PROFILE
    )
    LAST_RESULTS = results
    return np.concatenate([results.results[k]["y"] for k in range(N_CORES)], axis=0)
